# revision 3
# baseline (speedup 1.0000x reference)
"""Trainium2 Bass kernel for SAM-style decomposed rel-pos attention.

Problem: B=1, HW=2304 (48x48), NH=16 heads, DH=64, D=1024, f32 in/out.
  attn = softmax(q*scale @ k^T + rel_h[q,kh] + rel_w[q,kw]); out = attn @ v

Strategy (8 NeuronCores, SPMD, no collectives):
  - 2 heads per core (tensor-parallel over heads). Each core gets its
    128-channel slice of q/k/v; rel tables replicated.
  - Host prep: per-core transposed, bf16-cast Q^T (pre-scaled) and K^T;
    V with a ones-column appended (denominator falls out of the PV matmul);
    gathered rel tables Rh^T/Rw^T (x8 to cancel the q scale); one-hot
    expansion matrices Eh/Ew that fold the decomposed bias into the score
    matmul as extra contraction rows.
  - Device per head: S^T tiles (128k x 512q) = [Eh;K^T]^T @ [rel_h^T;Q^T*s]
    (contract 112) + Ew^T @ rel_w^T (contract 48), exp on ScalarE
    (no max subtraction needed: scores are O(1)), PV matmul
    out^T = V_aug^T @ P^T accumulated over k tiles, then normalize by the
    ones-column row and DMA out^T rows to DRAM. Host transposes back.
"""

import sys

sys.path.insert(0, "/opt/trn_rl_repo")

import numpy as np
import ml_dtypes

from concourse import bacc, mybir, tile
from concourse.bass_utils import run_bass_kernel_spmd

BF16 = mybir.dt.bfloat16
F32 = mybir.dt.float32
BF = ml_dtypes.bfloat16

H = 48
W = 48
HW = H * W          # 2304
DH = 64
NH = 16
N_CORES = 8
HPC = 2             # heads per core
KT = HW // 128      # 18 k tiles
QCHUNKS = [(0, 512), (512, 512), (1024, 512), (1536, 512), (2048, 256)]

_NC = None


def _build_nc():
    nc = bacc.Bacc(None, target_bir_lowering=False)

    q_t = nc.dram_tensor("q_t", [128, HW], BF16, kind="ExternalInput")
    k_t = nc.dram_tensor("k_t", [128, HW], BF16, kind="ExternalInput")
    v_til = nc.dram_tensor("v_til", [128, HPC * KT * 65], BF16, kind="ExternalInput")
    rh_t = nc.dram_tensor("rh_t", [64, HW], BF16, kind="ExternalInput")
    rw_t = nc.dram_tensor("rw_t", [64, HW], BF16, kind="ExternalInput")
    eh = nc.dram_tensor("eh", [64, HW], BF16, kind="ExternalInput")
    ew = nc.dram_tensor("ew", [48, HW], BF16, kind="ExternalInput")
    out_t = nc.dram_tensor("out_t", [128, HW], F32, kind="ExternalOutput")

    Exp = mybir.ActivationFunctionType.Exp

    with tile.TileContext(nc) as tc:
        with (
            tc.tile_pool(name="const", bufs=1) as cpool,
            tc.tile_pool(name="stack", bufs=2) as spool,
            tc.tile_pool(name="ptile", bufs=3) as ppool,
            tc.tile_pool(name="epil", bufs=2) as epool,
            tc.tile_pool(name="ps_s", bufs=3, space="PSUM") as ps_s,
            tc.tile_pool(name="ps_o", bufs=2, space="PSUM") as ps_o,
            tc.tile_pool(name="ps_rel", bufs=2, space="PSUM") as ps_rel,
            tc.tile_pool(name="ps_rb", bufs=1, space="PSUM") as ps_rb,
        ):
            # shared constants; rh/rw live at partitions 48:112 so the rel
            # matmul operands share the partition range of Q^T in the stack
            rh_sb = cpool.tile([128, HW], BF16, tag="rh")
            rw_sb = cpool.tile([128, HW], BF16, tag="rw")
            ew_sb = cpool.tile([48, HW], BF16, tag="ew")
            ones1 = cpool.tile([1, 64], BF16, tag="ones1")
            nc.sync.dma_start(rh_sb[64:128, :], rh_t[:, :])
            nc.sync.dma_start(rw_sb[64:128, :], rw_t[:, :])
            nc.sync.dma_start(ew_sb[:, :], ew[:, :])
            nc.gpsimd.memset(ones1[:], 1.0)

            for hh in range(HPC):
                c0, c1 = hh * 64, (hh + 1) * 64
                # stacks: rows 0:48 = bias block (Eh / rel_h^T), rows 48:64
                # zeroed, rows 64:128 = K^T / Q^T (matmul base-partition rule)
                lhsT = spool.tile([128, HW], BF16, tag="lhsT")
                rhs = spool.tile([128, HW], BF16, tag="rhs")
                relw = spool.tile([48, HW], BF16, tag="relw")
                vt = spool.tile([128, KT * 65], BF16, tag="vt")
                # eh rows 48:64 are host-zeros: rows 48:64 of both stacks
                # must contribute nothing to the contract-128 score matmul
                nc.sync.dma_start(lhsT[0:64, :], eh[:, :])
                nc.sync.dma_start(rhs[0:64, :], eh[:, :])
                nc.sync.dma_start(lhsT[64:128, :], k_t[c0:c1, :])
                nc.sync.dma_start(rhs[64:128, :], q_t[c0:c1, :])
                nc.sync.dma_start(vt[:, :], v_til[:, hh * KT * 65 : (hh + 1) * KT * 65])

                # rel_h^T[j, (h,w)] for block h -> rhs rows 0:48
                for h in range(H):
                    prel = ps_rel.tile([48, 48], F32, tag="prel")
                    nc.tensor.matmul(
                        prel[:],
                        rh_sb[64:128, h * 48 : h * 48 + 48],
                        rhs[64:128, h * 48 : h * 48 + 48],
                        start=True,
                        stop=True,
                    )
                    nc.vector.tensor_copy(rhs[0:48, h * 48 : h * 48 + 48], prel[:])

                # rel_w^T[j, (h,w)] for block w -> relw rows 0:48 (strided cols)
                rhs_q_byw = rhs[64:128, :].rearrange("p (h w) -> p w h", w=48)
                relw_byw = relw[:, :].rearrange("p (h w) -> p w h", w=48)
                for w in range(W):
                    prel = ps_rel.tile([48, 48], F32, tag="prel")
                    nc.tensor.matmul(
                        prel[:],
                        rw_sb[64:128, w * 48 : w * 48 + 48],
                        rhs_q_byw[:, w, :],
                        start=True,
                        stop=True,
                    )
                    nc.vector.tensor_copy(relw_byw[:, w, :], prel[:])

                for (q0, qn) in QCHUNKS:
                    o_ps = ps_o.tile([65, 512], F32, tag="o")
                    for kt in range(KT):
                        s_ps = ps_s.tile([128, 512], F32, tag="s")
                        nc.tensor.matmul(
                            s_ps[:, 0:qn],
                            lhsT[:, kt * 128 : (kt + 1) * 128],
                            rhs[:, q0 : q0 + qn],
                            start=True,
                            stop=False,
                        )
                        nc.tensor.matmul(
                            s_ps[:, 0:qn],
                            ew_sb[:, kt * 128 : (kt + 1) * 128],
                            relw[:, q0 : q0 + qn],
                            start=False,
                            stop=True,
                        )
                        p_sb = ppool.tile([128, 512], BF16, tag="p")
                        nc.scalar.activation(p_sb[:, 0:qn], s_ps[:, 0:qn], Exp)
                        nc.tensor.matmul(
                            o_ps[:, 0:qn],
                            vt[:, kt * 65 : (kt + 1) * 65],
                            p_sb[:, 0:qn],
                            start=(kt == 0),
                            stop=(kt == KT - 1),
                        )

                    # normalize: denom = row 64 of o_ps
                    den65 = epool.tile([65, 512], F32, tag="den65")
                    nc.vector.tensor_copy(den65[64:65, 0:qn], o_ps[64:65, 0:qn])
                    den0 = epool.tile([1, 512], F32, tag="den0")
                    nc.sync.dma_start(den0[0:1, 0:qn], den65[64:65, 0:qn])
                    rec0 = epool.tile([1, 512], F32, tag="rec0")
                    nc.vector.reciprocal(rec0[0:1, 0:qn], den0[0:1, 0:qn])
                    recb = epool.tile([1, 512], BF16, tag="recb")
                    nc.vector.tensor_copy(recb[0:1, 0:qn], rec0[0:1, 0:qn])
                    rb_ps = ps_rb.tile([64, 512], F32, tag="rb")
                    nc.tensor.matmul(
                        rb_ps[:, 0:qn], ones1[:], recb[0:1, 0:qn], start=True, stop=True
                    )
                    rb_sb = epool.tile([64, 512], F32, tag="rb_sb")
                    nc.vector.tensor_copy(rb_sb[:, 0:qn], rb_ps[:, 0:qn])
                    ot = epool.tile([64, 512], F32, tag="ot")
                    nc.vector.tensor_mul(ot[:, 0:qn], o_ps[0:64, 0:qn], rb_sb[:, 0:qn])
                    nc.sync.dma_start(out_t[c0:c1, q0 : q0 + qn], ot[:, 0:qn])

    nc.compile()
    return nc


def _get_nc():
    global _NC
    if _NC is None:
        _NC = _build_nc()
    return _NC


def _host_prep(q, k, v, rel_pos_h, rel_pos_w):
    q2 = np.asarray(q, np.float32).reshape(HW, NH * DH)
    k2 = np.asarray(k, np.float32).reshape(HW, NH * DH)
    v2 = np.asarray(v, np.float32).reshape(HW, NH * DH)
    rph = np.asarray(rel_pos_h, np.float32)
    rpw = np.asarray(rel_pos_w, np.float32)

    ar = np.arange(48)
    coords = ar[:, None] - ar[None, :] + 47
    # Rx_t[c, h*48+j] = rel_pos_x[h-j+47, c]; x8 cancels the 0.125 q scale
    rh_t = (8.0 * rph[coords].transpose(2, 0, 1).reshape(64, HW)).astype(BF)
    rw_t = (8.0 * rpw[coords].transpose(2, 0, 1).reshape(64, HW)).astype(BF)
    kk = np.arange(HW)
    eh = np.zeros((64, HW), np.float32)
    eh[:48] = kk[None, :] // 48 == ar[:, None]
    eh = eh.astype(BF)
    ew = (kk[None, :] % 48 == ar[:, None]).astype(BF)

    onecol = np.ones((HW, 1), np.float32)
    in_maps = []
    for c in range(N_CORES):
        sl = slice(c * 128, (c + 1) * 128)
        qs = (q2[:, sl].T * 0.125).astype(BF)
        ks = k2[:, sl].T.astype(BF)
        vparts = []
        for hh in range(HPC):
            vh = v2[:, c * 128 + hh * 64 : c * 128 + (hh + 1) * 64]
            va = np.concatenate([vh, onecol], axis=1)            # (HW, 65)
            vparts.append(va.reshape(KT, 128, 65).transpose(1, 0, 2).reshape(128, KT * 65))
        v_til = np.concatenate(vparts, axis=1).astype(BF)        # (128, 2*18*65)
        in_maps.append(
            dict(q_t=qs, k_t=ks, v_til=v_til, rh_t=rh_t, rw_t=rw_t, eh=eh, ew=ew)
        )
    return in_maps


def _assemble(results):
    cols = [np.asarray(r["out_t"], np.float32).T for r in results]  # (HW, 128) each
    return np.concatenate(cols, axis=1).reshape(1, H, W, NH * DH)


def kernel(q, k, v, rel_pos_h, rel_pos_w):
    nc = _get_nc()
    in_maps = _host_prep(q, k, v, rel_pos_h, rel_pos_w)
    res = run_bass_kernel_spmd(nc, in_maps, core_ids=list(range(N_CORES)))
    return _assemble(res.results)
